# revision 16
# baseline (speedup 1.0000x reference)
"""DGCNN-alt Trainium2 kernel: 8-core data-parallel (4 graphs/core).

Self-contained: builds a Bass/Tile kernel, shards inputs across 8 NeuronCores,
runs via PJRT (axon), gathers the full [32, 40] output.

v2: exact fp32 kNN top-k (max/max_index/match_replace — no quantized keys),
all constants packed into one device-resident buffer so steady-state runs
only upload pos (384 KB) and download the [32,40] logits.
"""
import sys
sys.path.insert(0, '/opt/trn_rl_repo')
import hashlib
import numpy as np

import concourse.bass as bass
from concourse import bacc
import concourse.mybir as mybir
from concourse.tile import TileContext
from concourse.bass import IndirectOffsetOnAxis

f32 = mybir.dt.float32
f32r = mybir.dt.float32r
u32 = mybir.dt.uint32
AF = mybir.ActivationFunctionType
ALU = mybir.AluOpType

# ---- problem constants ----
B, N, D, K = 32, 1024, 3, 20
GPC = 4                 # graphs per core
NCORES = 8
EPS = 1e-5
NEDGE = N * K           # 20480 edges/graph
M_EDGES = float(B * NEDGE)   # BN denominator over the full batch
NEG = -3.0e38
KC = 24                 # top-k candidates extracted per chunk (use first 20)

# ---- packed-constant layout (name -> (partitions, free)) ----
_PACK = [
    ("Rsel", 128, 512),
    ("ident", 128, 128),
    ("W1b", D, 64),
    ("W1d", D, 64),
    ("W2", 128, 64),
    ("W3", 128, 64),
    ("Wc2d", 64, 128),
    ("Wc2b", 64, 128),
    ("WlX1", 64, 1024),
    ("WlX2", 128, 1024),
    ("Wm1", 128, 4096),   # streamed from DRAM at use time (not resident)
    ("Wm2", 128, 1024),
    ("Wm3", 128, 80),
    ("b3c", 128, 1),
    ("bc2c", 128, 1),
    ("blc", 128, 8),
    ("bm1c", 128, 4),
    ("bm2c", 128, 2),
    ("bm3c", 40, 1),
    ("onesr", 1, 1024),
    ("g1r", 1, 64),
    ("be1r", 1, 64),
    ("g2r", 1, 64),
    ("be2r", 1, 64),
]
_OFFS = {}
_L = 0
for _n, _p, _f in _PACK:
    _OFFS[_n] = _L
    _L += _p * _f
PACK_LEN = _L

# tiles loaded as f32r (weights feeding fast PE matmuls)
_F32R = {"Rsel", "W1b", "W2", "W3", "Wc2d", "Wc2b", "WlX1", "WlX2",
         "Wm1", "Wm2", "Wm3"}


def _build():
    nc = bacc.Bacc()

    # ---------------- I/O ----------------
    pos4 = nc.dram_tensor("pos4", [GPC * N, D], f32, kind="ExternalInput")
    cpack = nc.dram_tensor("cpack", [1, PACK_LEN], f32, kind="ExternalInput")
    # replicated output: every core holds all 8 cores' [40 x GPC] blocks
    out_t = nc.dram_tensor("out", [1, NCORES * 40 * GPC], f32,
                           kind="ExternalOutput")
    cc3_in = nc.dram_tensor("cc3_in", [1, 40 * GPC], f32)
    cc3_out = nc.dram_tensor("cc3_out", [1, NCORES * 40 * GPC], f32,
                             addr_space="Shared")

    # internal DRAM
    v2d = [nc.dram_tensor(f"v2d_{g}", [N, 128], f32) for g in range(GPC)]
    cc1_in = nc.dram_tensor("cc1_in", [1, 128], f32)
    cc1_out = nc.dram_tensor("cc1_out", [1, 128], f32, addr_space="Shared")
    cc2_in = nc.dram_tensor("cc2_in", [1, 128], f32)
    cc2_out = nc.dram_tensor("cc2_out", [1, 128], f32, addr_space="Shared")
    rg = [list(range(NCORES))]

    with TileContext(nc) as tc:
        with tc.tile_pool(name="cst", bufs=1) as cst, \
             tc.tile_pool(name="big", bufs=1) as big, \
             tc.tile_pool(name="wrk", bufs=2) as wrk, \
             tc.tile_pool(name="sm", bufs=1) as sm, \
             tc.tile_pool(name="psA", bufs=1, space="PSUM") as psA, \
             tc.tile_pool(name="psB", bufs=2, space="PSUM") as psB, \
             tc.tile_pool(name="psC", bufs=2, space="PSUM") as psC:

            # ---------- load packed constants ----------
            ct = {}
            for name, p, f in _PACK:
                if name == "Wm1":
                    continue              # streamed at use time
                dt = f32r if name in _F32R else f32
                t = cst.tile([p, f], dt, name=name)
                src = cpack[0:1, _OFFS[name]:_OFFS[name] + p * f] \
                    .rearrange("o (p f) -> (o p) f", p=p)
                if dt is f32r:
                    src = src.bitcast(f32r)
                nc.sync.dma_start(out=t, in_=src)
                ct[name] = t

            def wm1_slice(dst, col0, ncol):
                src = cpack[0:1, _OFFS["Wm1"]:_OFFS["Wm1"] + 128 * 4096] \
                    .rearrange("o (p f) -> (o p) f", p=128)[:, col0:col0 + ncol]
                nc.sync.dma_start(out=dst, in_=src.bitcast(f32r))
            RT, idT = ct["Rsel"], ct["ident"]
            W1bT, W1dT = ct["W1b"], ct["W1d"]
            W2T, W3T = ct["W2"], ct["W3"]
            Wc2dT, Wc2bT = ct["Wc2d"], ct["Wc2b"]
            WlX1T, WlX2T = ct["WlX1"], ct["WlX2"]
            b3cT, bc2cT, blcT = ct["b3c"], ct["bc2c"], ct["blc"]
            g1rT, be1rT = ct["g1r"], ct["be1r"]
            g2rT, be2rT = ct["g2r"], ct["be2r"]
            onesr_d = cpack[0:1, _OFFS["onesr"]:_OFFS["onesr"] + N]

            ones3 = cst.tile([D, 1], f32, name="ones3")
            nc.vector.memset(ones3, 1.0)
            ones64 = cst.tile([64, 1], f32, name="ones64")
            nc.vector.memset(ones64, 1.0)

            bn1sc = cst.tile([128, 1], f32, name="bn1sc")
            bn1sh = cst.tile([128, 1], f32, name="bn1sh")
            bn2sc = cst.tile([128, 1], f32, name="bn2sc")
            bn2sh = cst.tile([128, 1], f32, name="bn2sh")

            # per-graph persistent (small) tiles
            posje = [big.tile([128, 480], f32, name=f"posje{g}") for g in range(GPC)]
            u1s = [big.tile([128, 512], f32r, name=f"u1s{g}") for g in range(GPC)]
            idx1s = [big.tile([128, 8 * KC], u32, name=f"idx1s{g}") for g in range(GPC)]
            idx2s = [big.tile([128, 8 * KC], u32, name=f"idx2s{g}") for g in range(GPC)]
            x1r = [big.tile([64, 1024], f32r, name=f"x1r{g}") for g in range(GPC)]
            x2r = [big.tile([128, 1024], f32r, name=f"x2r{g}") for g in range(GPC)]
            x1f = [big.tile([64, 1024], f32, name=f"x1f{g}") for g in range(GPC)]
            pooled4 = cst.tile([128, 32], f32, name="pooled4")
            s1acc = cst.tile([128, GPC], f32, name="s1acc")
            s1sq = cst.tile([128, GPC], f32, name="s1sq")
            s1pacc = cst.tile([128, GPC], f32, name="s1pacc")
            s2sq = cst.tile([128, GPC], f32, name="s2sq")
            for st_ in (s1acc, s1sq, s1pacc, s2sq):
                nc.vector.memset(st_, 0.0)

            P4a = [sm.tile([5, N], f32, name=f"P4a{g}", tag="P4a") for g in range(GPC)]
            P4b = [sm.tile([5, N], f32, name=f"P4b{g}", tag="P4b") for g in range(GPC)]

            def topk_chunks(src65a, src65b, idxout, extra_add):
                # exact fp32 top-24 (values+indices) per 128-row chunk of the
                # negated distance matrix; top-20 of those are the kNN set.
                for c in range(8):
                    ps = psA.tile([128, N], f32, name="psd", tag="psa")
                    nc.tensor.matmul(ps[:, 0:512], src65a[:, 128 * c:128 * (c + 1)],
                                     src65b[:, 0:512], start=True, stop=True)
                    nc.tensor.matmul(ps[:, 512:1024], src65a[:, 128 * c:128 * (c + 1)],
                                     src65b[:, 512:1024], start=True, stop=True)
                    t = wrk.tile([128, N], f32, name="tneg", tag="dwork")
                    nc.scalar.activation(t, ps, AF.Copy, scale=2.0)      # -d
                    v24 = wrk.tile([128, 24], f32, name="v24", tag="t24")
                    scr = wrk.tile([128, N], f32, name="scr", tag="scr")
                    o = KC * c
                    nc.vector.max(out=v24[:, 0:8], in_=t)
                    nc.vector.max_index(out=idxout[:, o:o + 8],
                                        in_max=v24[:, 0:8], in_values=t)
                    nc.vector.match_replace(out=scr, in_to_replace=v24[:, 0:8],
                                            in_values=t, imm_value=NEG)
                    nc.vector.max(out=v24[:, 8:16], in_=scr)
                    nc.vector.max_index(out=idxout[:, o + 8:o + 16],
                                        in_max=v24[:, 8:16], in_values=scr)
                    nc.vector.match_replace(out=scr, in_to_replace=v24[:, 8:16],
                                            in_values=scr, imm_value=NEG)
                    nc.vector.max(out=v24[:, 16:24], in_=scr)
                    nc.vector.max_index(out=idxout[:, o + 16:o + 24],
                                        in_max=v24[:, 16:24], in_values=scr)
                if extra_add:
                    nc.vector.tensor_scalar(idxout, idxout, extra_add,
                                            scalar2=None, op0=ALU.add)

            # slice sl in [0,40): (c, q) = divmod(sl, 5); ranks 4q..4q+3 of chunk c
            # all MLP compute on partitions 0-63; groups of 2 slices -> [64,1024] psum
            def mat_h1(g, mode):
                for bt in range(5):
                    pst = psB.tile([96, 128], f32, name="pst", tag="psb")
                    nc.tensor.transpose(pst, posje[g][:, 96 * bt:96 * (bt + 1)], idT)
                    xtmp = wrk.tile([96, 128], f32r, name="xtmp", tag="xtmp")
                    nc.scalar.activation(xtmp, pst, AF.Copy)
                    piece = wrk.tile([3, 4096], f32r, name="piece", tag="piece", bufs=1)
                    for r3 in range(3):
                        nc.sync.dma_start(
                            out=piece[r3:r3 + 1, :].rearrange("o (t p) -> o t p", p=128),
                            in_=xtmp[r3:96:3, :])
                    for j in range(4 * bt, 4 * bt + 4):   # 1024-edge groups
                        ph = psC.tile([64, 1024], f32, name="ph", tag="psc")
                        for q_ in range(2):
                            sl = 2 * j + q_
                            cch = sl // 5
                            pcol = 512 * (sl - 8 * bt)
                            po = ph[:, 512 * q_:512 * (q_ + 1)]
                            nc.tensor.matmul(po, W1bT,
                                             piece[:, pcol:pcol + 512],
                                             start=True, stop=False)
                            nc.tensor.matmul(po, u1s[g][:, 64 * cch:64 * cch + 64],
                                             RT, start=False, stop=True)
                        if mode == 1:
                            sac = wrk.tile([64, 2], f32, name="sac", tag="sac")
                            d1 = wrk.tile([64, 1024], f32, name="d1", tag="d1")
                            nc.scalar.activation(d1, ph, AF.Copy,
                                                 accum_out=sac[:, 0:1])
                            d2 = wrk.tile([64, 1024], f32, name="d2", tag="d2")
                            nc.scalar.activation(d2, ph, AF.Square,
                                                 accum_out=sac[:, 1:2])
                            if j == 0:
                                nc.vector.tensor_copy(s1acc[0:64, g:g + 1], sac[:, 0:1])
                                nc.vector.tensor_copy(s1sq[0:64, g:g + 1], sac[:, 1:2])
                            else:
                                nc.vector.tensor_tensor(s1acc[0:64, g:g + 1],
                                                        s1acc[0:64, g:g + 1],
                                                        sac[:, 0:1], op=ALU.add)
                                nc.vector.tensor_tensor(s1sq[0:64, g:g + 1],
                                                        s1sq[0:64, g:g + 1],
                                                        sac[:, 1:2], op=ALU.add)
                        else:
                            sacp = wrk.tile([64, 1], f32, name="sacp", tag="sacp")
                            h1p = wrk.tile([64, 1024], f32r, name="h1p", tag="h1p")
                            nc.scalar.activation(h1p, ph, AF.Relu,
                                                 scale=bn1sc[0:64, 0:1],
                                                 bias=bn1sh[0:64, 0:1],
                                                 accum_out=sacp)
                            if mode == 2:
                                if j == 0:
                                    nc.vector.tensor_copy(s1pacc[0:64, g:g + 1], sacp)
                                else:
                                    nc.vector.tensor_tensor(s1pacc[0:64, g:g + 1],
                                                            s1pacc[0:64, g:g + 1],
                                                            sacp, op=ALU.add)
                            ph2 = psC.tile([64, 1024], f32, name="ph2", tag="psc")
                            nc.tensor.matmul(ph2[:, 0:512], W2T[0:64, :],
                                             h1p[:, 0:512], start=True, stop=True)
                            nc.tensor.matmul(ph2[:, 512:1024], W2T[0:64, :],
                                             h1p[:, 512:1024], start=True, stop=True)
                            if mode == 2:
                                sq2a = wrk.tile([64, 1], f32, name="sq2a", tag="sq2a")
                                d3 = wrk.tile([64, 1024], f32, name="d3", tag="d1")
                                nc.scalar.activation(d3, ph2, AF.Square,
                                                     accum_out=sq2a)
                                if j == 0:
                                    nc.vector.tensor_copy(s2sq[0:64, g:g + 1], sq2a)
                                else:
                                    nc.vector.tensor_tensor(s2sq[0:64, g:g + 1],
                                                            s2sq[0:64, g:g + 1],
                                                            sq2a, op=ALU.add)
                            else:
                                h2p = wrk.tile([64, 1024], f32r, name="h2p", tag="h1p")
                                nc.scalar.activation(h2p, ph2, AF.Relu,
                                                     scale=bn2sc[0:64, 0:1],
                                                     bias=bn2sh[0:64, 0:1])
                                ph3 = psC.tile([64, 1024], f32, name="ph3", tag="psc")
                                nc.tensor.matmul(ph3[:, 0:512], W3T[0:64, :],
                                                 h2p[:, 0:512], start=True, stop=True)
                                nc.tensor.matmul(ph3[:, 512:1024], W3T[0:64, :],
                                                 h2p[:, 512:1024],
                                                 start=True, stop=True)
                                h3t = wrk.tile([64, 1024], f32, name="h3t", tag="d2")
                                nc.scalar.activation(h3t, ph3, AF.Identity,
                                                     bias=b3cT[0:64, 0:1])
                                # streamed x1 partial reduce over the 2 slices
                                for q_ in range(2):
                                    sl = 2 * j + q_
                                    cch = sl // 5
                                    xcol = slice(128 * cch, 128 * (cch + 1))
                                    red = h3t[:, 512 * q_:512 * (q_ + 1)].rearrange(
                                        "z (rr p) -> z p rr", p=128)
                                    if sl % 5 == 0:
                                        nc.vector.tensor_reduce(
                                            out=x1f[g][:, xcol], in_=red,
                                            op=ALU.max, axis=mybir.AxisListType.X)
                                    else:
                                        xtm = wrk.tile([64, 128], f32, name="xtm",
                                                       tag="xtm")
                                        nc.vector.tensor_reduce(
                                            out=xtm, in_=red,
                                            op=ALU.max, axis=mybir.AxisListType.X)
                                        nc.vector.tensor_tensor(
                                            x1f[g][:, xcol], x1f[g][:, xcol],
                                            xtm, op=ALU.max)

            # ================= phase 1: kNN1, gathers, u1, stats1 =================
            for g in range(GPC):
                pg = pos4[N * g:N * (g + 1), :].rearrange("n c -> c n")
                nc.sync.dma_start(out=P4a[g][0:3, :], in_=pg)
                nc.sync.dma_start(out=P4b[g][0:3, :], in_=pg)
                nc.sync.dma_start(out=P4a[g][3:4, :], in_=onesr_d)
                nc.sync.dma_start(out=P4b[g][4:5, :], in_=onesr_d)
                psq = sm.tile([D, N], f32, name="psq", tag="psq")
                nc.scalar.activation(psq, P4a[g][0:3, :], AF.Square)
                ps1 = psA.tile([1, N], f32, name="ps1", tag="psa")
                nc.tensor.matmul(ps1[:, 0:512], ones3, psq[:, 0:512],
                                 start=True, stop=True)
                nc.tensor.matmul(ps1[:, 512:1024], ones3, psq[:, 512:1024],
                                 start=True, stop=True)
                msqrow = sm.tile([1, N], f32, name="msqrow", tag="msqrow")
                nc.scalar.activation(msqrow, ps1, AF.Copy, scale=-0.5)
                nc.sync.dma_start(out=P4b[g][3:4, :], in_=msqrow)
                nc.sync.dma_start(out=P4a[g][4:5, :], in_=msqrow)
                topk_chunks(P4a[g], P4b[g], idx1s[g], 1024 * g if g else None)

                for c in range(8):
                    pu = psB.tile([128, 64], f32, name="pu", tag="psb")
                    nc.tensor.matmul(pu, P4a[g][0:3, 128 * c:128 * (c + 1)],
                                     W1dT, start=True, stop=True)
                    nc.scalar.activation(u1s[g][:, 64 * c:64 * (c + 1)], pu, AF.Copy)

                for t in range(160):
                    c, r = divmod(t, K)
                    nc.gpsimd.indirect_dma_start(
                        out=posje[g][:, 3 * t:3 * t + 3], out_offset=None,
                        in_=pos4.ap(),
                        in_offset=IndirectOffsetOnAxis(
                            ap=idx1s[g][:, KC * c + r:KC * c + r + 1], axis=0))
                mat_h1(g, 1)

            # ================= AllReduce #1 =================
            def bn_allreduce(s_a, s_b, cc_in_t, cc_out_t, grow, berow, scol, shcol):
                stot = sm.tile([128, 2], f32, name="stot", tag="stot")
                nc.vector.tensor_reduce(out=stot[:, 0:1], in_=s_a,
                                        op=ALU.add, axis=mybir.AxisListType.X)
                nc.vector.tensor_reduce(out=stot[:, 1:2], in_=s_b,
                                        op=ALU.add, axis=mybir.AxisListType.X)
                pack = sm.tile([1, 128], f32, name="pack", tag="pack")
                nc.sync.dma_start(out=pack[:, 0:64], in_=stot[0:64, 0:1])
                nc.sync.dma_start(out=pack[:, 64:128], in_=stot[0:64, 1:2])
                nc.sync.dma_start(out=cc_in_t[:, :], in_=pack)
                nc.gpsimd.collective_compute(
                    "AllReduce", ALU.add, replica_groups=rg,
                    ins=[cc_in_t.ap().opt()], outs=[cc_out_t.ap().opt()])
                red = sm.tile([1, 128], f32, name="red", tag="red")
                nc.sync.dma_start(out=red, in_=cc_out_t[:, :])
                mean = sm.tile([1, 64], f32, name="mean", tag="mean")
                nc.vector.tensor_scalar(mean, red[:, 0:64], 1.0 / M_EDGES,
                                        scalar2=None, op0=ALU.mult)
                var = sm.tile([1, 64], f32, name="var", tag="var")
                nc.vector.tensor_scalar(var, red[:, 64:128], 1.0 / M_EDGES,
                                        scalar2=None, op0=ALU.mult)
                msq = sm.tile([1, 64], f32, name="msq", tag="msq")
                nc.vector.tensor_tensor(msq, mean, mean, op=ALU.mult)
                nc.vector.tensor_tensor(var, var, msq, op=ALU.subtract)
                nc.vector.tensor_scalar(var, var, EPS, scalar2=None, op0=ALU.add)
                rcp = sm.tile([1, 64], f32, name="rcp", tag="rcp")
                nc.vector.reciprocal(rcp, var)
                nc.scalar.activation(rcp, rcp, AF.Sqrt)
                scrow = sm.tile([1, 64], f32, name="scrow", tag="scrow")
                nc.vector.tensor_tensor(scrow, grow, rcp, op=ALU.mult)
                shrow = sm.tile([1, 64], f32, name="shrow", tag="shrow")
                nc.vector.tensor_tensor(shrow, scrow, mean, op=ALU.mult)
                nc.vector.tensor_tensor(shrow, berow, shrow, op=ALU.subtract)
                nc.sync.dma_start(out=scol[0:64, :], in_=scrow)
                nc.sync.dma_start(out=scol[64:128, :], in_=scrow)
                nc.sync.dma_start(out=shcol[0:64, :], in_=shrow)
                nc.sync.dma_start(out=shcol[64:128, :], in_=shrow)

            bn_allreduce(s1acc, s1sq, cc1_in, cc1_out, g1rT, be1rT, bn1sc, bn1sh)

            # ================= phase 2: stats2 =================
            for g in range(GPC):
                mat_h1(g, 2)
            s1pr = sm.tile([64, GPC], f32r, name="s1pr", tag="s1pr")
            nc.vector.tensor_copy(s1pr, s1pacc[0:64, :])
            ps2s = psB.tile([64, GPC], f32, name="ps2s", tag="psb")
            nc.tensor.matmul(ps2s, W2T[0:64, :], s1pr, start=True, stop=True)
            s2sum = sm.tile([128, GPC], f32, name="s2sum", tag="s2sum")
            nc.vector.memset(s2sum, 0.0)
            nc.scalar.activation(s2sum[0:64, :], ps2s, AF.Copy)

            bn_allreduce(s2sum, s2sq, cc2_in, cc2_out, g2rT, be2rT, bn2sc, bn2sh)

            # ====== phase 3+4 per graph: h3 -> x1; knn2; conv2; lin ======
            for g in range(GPC):
                mat_h1(g, 3)
                nc.vector.tensor_copy(x1r[g], x1f[g])

                # v2 node-major -> DRAM
                v2s = sm.tile([128, 1024], f32, name="v2s", tag="v2s")
                for c in range(8):
                    pv = psB.tile([128, 128], f32, name="pv", tag="psb")
                    nc.tensor.matmul(pv, x1r[g][:, 128 * c:128 * (c + 1)], Wc2bT,
                                     start=True, stop=True)
                    nc.scalar.activation(v2s[:, 128 * c:128 * (c + 1)], pv, AF.Copy)
                nc.sync.dma_start(
                    out=v2d[g].ap().rearrange("(c p) d -> p c d", p=128),
                    in_=v2s.rearrange("p (c d) -> p c d", c=8))

                # kNN2
                x1q = sm.tile([64, 1024], f32, name="x1q", tag="v2s")
                nc.scalar.activation(x1q, x1r[g].bitcast(f32), AF.Square)
                ps2 = psA.tile([1, N], f32, name="ps2", tag="psa")
                nc.tensor.matmul(ps2[:, 0:512], ones64, x1q[:, 0:512],
                                 start=True, stop=True)
                nc.tensor.matmul(ps2[:, 512:1024], ones64, x1q[:, 512:1024],
                                 start=True, stop=True)
                X65a = sm.tile([66, 1024], f32, name="X65a", tag="X65a")
                X65b = sm.tile([66, 1024], f32, name="X65b", tag="X65b")
                nc.scalar.activation(X65a[0:64, :], x1r[g].bitcast(f32), AF.Copy)
                nc.scalar.activation(X65b[0:64, :], x1r[g].bitcast(f32), AF.Copy)
                nc.sync.dma_start(out=X65a[64:65, :], in_=onesr_d)
                nc.sync.dma_start(out=X65b[65:66, :], in_=onesr_d)
                msq2row = sm.tile([1, N], f32, name="msq2row", tag="msqrow")
                nc.scalar.activation(msq2row, ps2, AF.Copy, scale=-0.5)
                nc.sync.dma_start(out=X65b[64:65, :], in_=msq2row)
                nc.sync.dma_start(out=X65a[65:66, :], in_=msq2row)
                # swap of the msq/ones rows between a and b halves: a carries
                # [x;1;msq], b carries [x;msq;1] so a.T @ b = -0.5 d
                topk_chunks(X65a, X65b, idx2s[g], None)

                # conv2 gather + max
                maxv2 = sm.tile([128, 1024], f32, name="maxv2", tag="v2s")
                for c in range(8):
                    gdest = wrk.tile([128, K * 128], f32, name="gdest", tag="gdest", bufs=1)
                    for r in range(K):
                        nc.gpsimd.indirect_dma_start(
                            out=gdest[:, 128 * r:128 * (r + 1)], out_offset=None,
                            in_=v2d[g].ap(),
                            in_offset=IndirectOffsetOnAxis(
                                ap=idx2s[g][:, KC * c + r:KC * c + r + 1], axis=0))
                    nc.vector.tensor_reduce(
                        out=maxv2[:, 128 * c:128 * (c + 1)],
                        in_=gdest.rearrange("p (r d) -> p d r", r=K),
                        op=ALU.max, axis=mybir.AxisListType.X)
                mvT = sm.tile([128, 1024], f32, name="mvT", tag="X65b")
                for c in range(8):
                    pt2 = psB.tile([128, 128], f32, name="pt2", tag="psb")
                    nc.tensor.transpose(pt2, maxv2[:, 128 * c:128 * (c + 1)], idT)
                    nc.scalar.activation(mvT[:, 128 * c:128 * (c + 1)], pt2, AF.Copy)
                u2s = sm.tile([128, 1024], f32, name="u2s", tag="X65a")
                for h2_ in range(2):
                    pu2 = psB.tile([128, 512], f32, name="pu2", tag="psb")
                    nc.tensor.matmul(pu2, Wc2dT, x1r[g][:, 512 * h2_:512 * (h2_ + 1)],
                                     start=True, stop=True)
                    nc.scalar.activation(u2s[:, 512 * h2_:512 * (h2_ + 1)], pu2,
                                         AF.Identity, bias=bc2cT[:, 0:1])
                nc.vector.tensor_tensor(x2r[g], u2s, mvT, op=ALU.add)

                # lin + maxpool
                for c in range(8):
                    pm = wrk.tile([128, 2], f32, name="pm", tag="pm")
                    for s_ in range(2):
                        pl = psB.tile([128, 512], f32, name="pl", tag="psb")
                        nc.tensor.matmul(pl, WlX1T[:, 128 * c:128 * (c + 1)],
                                         x1r[g][:, 512 * s_:512 * (s_ + 1)],
                                         start=True, stop=False)
                        nc.tensor.matmul(pl, WlX2T[:, 128 * c:128 * (c + 1)],
                                         x2r[g][:, 512 * s_:512 * (s_ + 1)],
                                         start=False, stop=True)
                        nc.vector.tensor_reduce(out=pm[:, s_:s_ + 1], in_=pl,
                                                op=ALU.max, axis=mybir.AxisListType.X)
                    nc.vector.tensor_tensor(pooled4[:, 4 * c + g:4 * c + g + 1],
                                            pm[:, 0:1], pm[:, 1:2], op=ALU.max)

            pooled4r = cst.tile([128, 32], f32r, name="pooled4r")
            for c in range(8):
                nc.vector.tensor_tensor(pooled4r[:, 4 * c:4 * (c + 1)],
                                        pooled4[:, 4 * c:4 * (c + 1)],
                                        blcT[:, c:c + 1].to_broadcast([128, GPC]),
                                        op=ALU.add)

            # ============ head MLP ============
            Wm2T, Wm3T = ct["Wm2"], ct["Wm3"]
            bm1cT, bm2cT, bm3cT = ct["bm1c"], ct["bm2c"], ct["bm3c"]

            hm1 = cst.tile([128, 4 * GPC], f32r, name="hm1")
            for cc in range(4):
                phm = psB.tile([128, GPC], f32, name="phm", tag="psb")
                for kk in range(8):
                    wslc = wrk.tile([128, 128], f32r, name="wslc", tag="wslc")
                    wm1_slice(wslc, 512 * kk + 128 * cc, 128)
                    nc.tensor.matmul(phm, wslc,
                                     pooled4r[:, 4 * kk:4 * (kk + 1)],
                                     start=(kk == 0), stop=(kk == 7))
                nc.scalar.activation(hm1[:, GPC * cc:GPC * (cc + 1)], phm, AF.Relu,
                                     bias=bm1cT[:, cc:cc + 1])
            hm2 = cst.tile([128, 2 * GPC], f32r, name="hm2")
            for cc in range(2):
                phm2 = psB.tile([128, GPC], f32, name="phm2", tag="psb")
                for kk in range(4):
                    nc.tensor.matmul(phm2,
                                     Wm2T[:, 256 * kk + 128 * cc:
                                          256 * kk + 128 * (cc + 1)],
                                     hm1[:, GPC * kk:GPC * (kk + 1)],
                                     start=(kk == 0), stop=(kk == 3))
                nc.scalar.activation(hm2[:, GPC * cc:GPC * (cc + 1)], phm2, AF.Relu,
                                     bias=bm2cT[:, cc:cc + 1])
            pho = psB.tile([40, GPC], f32, name="pho", tag="psb")
            for kk in range(2):
                nc.tensor.matmul(pho, Wm3T[:, 40 * kk:40 * (kk + 1)],
                                 hm2[:, GPC * kk:GPC * (kk + 1)],
                                 start=(kk == 0), stop=(kk == 1))
            outsb = cst.tile([40, GPC], f32, name="outsb")
            nc.scalar.activation(outsb, pho, AF.Identity, bias=bm3cT[:, 0:1])
            # all-gather the per-core [40 x GPC] logits so the host fetches
            # one replicated shard instead of 8
            nc.sync.dma_start(
                out=cc3_in[0:1, :].rearrange("o (r c) -> (o r) c", r=40),
                in_=outsb)
            nc.gpsimd.collective_compute(
                "AllGather", ALU.bypass, replica_groups=rg,
                ins=[cc3_in.ap().opt()], outs=[cc3_out.ap().opt()])
            gout = cst.tile([40, NCORES * GPC], f32, name="gout")
            gview = "o (c r k) -> (o r) c k"
            sview = "p (c k) -> p c k"
            nc.sync.dma_start(
                out=gout.rearrange(sview, c=NCORES),
                in_=cc3_out[0:1, :].rearrange(gview, c=NCORES, r=40))
            nc.sync.dma_start(
                out=out_t[0:1, :].rearrange(gview, c=NCORES, r=40),
                in_=gout.rearrange(sview, c=NCORES))

    nc.compile()
    return nc


# ---------------- host wrapper ----------------
_CACHE = {}


def _pack_consts(W1, b1, g1, be1, W2, b2, g2, be2, W3, b3, Wc2, bc2,
                 Wl, bl, Wm1, bm1, Wm2, bm2, Wm3, bm3):
    f = np.float32
    W1 = np.asarray(W1, f); W3_ = np.asarray(W3, f); Wc2 = np.asarray(Wc2, f)
    Wl = np.asarray(Wl, f); Wm1 = np.asarray(Wm1, f); Wm2 = np.asarray(Wm2, f)
    Wm3 = np.asarray(Wm3, f)
    vals = dict(
        Rsel=np.tile(np.eye(128, dtype=f), (1, 4)),
        ident=np.eye(128, dtype=f),
        W1b=W1[3:6].copy(), W1d=(W1[0:3] - W1[3:6]).copy(),
        W2=np.concatenate([np.asarray(W2, f)] * 2, 0),
        W3=np.concatenate([W3_] * 2, 0),
        Wc2d=(Wc2[0:64] - Wc2[64:128]).copy(), Wc2b=Wc2[64:128].copy(),
        WlX1=Wl[0:64].copy(), WlX2=Wl[64:192].copy(),
        Wm1=Wm1.reshape(8, 128, 512).transpose(1, 0, 2).reshape(128, 4096).copy(),
        Wm2=Wm2.reshape(4, 128, 256).transpose(1, 0, 2).reshape(128, 1024).copy(),
        Wm3=Wm3.reshape(2, 128, 40).transpose(1, 0, 2).reshape(128, 80).copy(),
        b3c=np.tile(np.asarray(b3, f), 2).reshape(128, 1),
        bc2c=np.asarray(bc2, f).reshape(128, 1),
        blc=np.asarray(bl, f).reshape(8, 128).T.copy(),
        bm1c=np.asarray(bm1, f).reshape(4, 128).T.copy(),
        bm2c=np.asarray(bm2, f).reshape(2, 128).T.copy(),
        bm3c=np.asarray(bm3, f).reshape(40, 1),
        onesr=np.ones((1, 1024), f),
        g1r=np.asarray(g1, f).reshape(1, 64),
        be1r=np.asarray(be1, f).reshape(1, 64),
        g2r=np.asarray(g2, f).reshape(1, 64),
        be2r=np.asarray(be2, f).reshape(1, 64),
    )
    pack = np.empty((1, PACK_LEN), f)
    for name, p, fr in _PACK:
        v = np.ascontiguousarray(vals[name], f)
        assert v.shape == (p, fr), (name, v.shape)
        pack[0, _OFFS[name]:_OFFS[name] + p * fr] = v.reshape(-1)
    return pack


def _get_session():
    if "sess" in _CACHE:
        return _CACHE["sess"]
    import jax
    from concourse.bass2jax import (install_neuronx_cc_hook, _bass_exec_p,
                                    partition_id_tensor)
    from jax.sharding import Mesh, PartitionSpec, NamedSharding
    from jax.experimental.shard_map import shard_map

    nc = _build()
    install_neuronx_cc_hook()
    partition_name = nc.partition_id_tensor.name if nc.partition_id_tensor else None
    in_names, out_names, out_avals, zero_outs = [], [], [], []
    for alloc in nc.m.functions[0].allocations:
        if not isinstance(alloc, mybir.MemoryLocationSet):
            continue
        name = alloc.memorylocations[0].name
        if alloc.kind == "ExternalInput":
            if name != partition_name:
                in_names.append(name)
        elif alloc.kind == "ExternalOutput":
            out_names.append(name)
            shape = tuple(alloc.tensor_shape)
            dtype = mybir.dt.np(alloc.dtype)
            out_avals.append(jax.core.ShapedArray(shape, dtype))
            zero_outs.append(np.zeros(shape, dtype))
    assert in_names == ["pos4", "cpack"], in_names
    assert out_names == ["out"], out_names
    n_params = len(in_names)
    all_in = list(in_names) + list(out_names)
    if partition_name is not None:
        all_in.append(partition_name)

    def _body(*args):
        operands = list(args)
        if partition_name is not None:
            operands.append(partition_id_tensor())
        return tuple(_bass_exec_p.bind(
            *operands, out_avals=tuple(out_avals), in_names=tuple(all_in),
            out_names=tuple(out_names), lowering_input_output_aliases=(),
            sim_require_finite=True, sim_require_nnan=True, nc=nc))

    devices = jax.devices()[:NCORES]
    mesh = Mesh(np.asarray(devices), ("core",))
    P = PartitionSpec

    def _full(pos_flat, cpack_all, zeros):
        # out is replicated across cores by the in-kernel AllGather; no jax
        # ops allowed here (neuronx hook only lowers the bass custom call)
        outs = shard_map(_body, mesh=mesh,
                         in_specs=(P("core"),) * 3,
                         out_specs=(P(),),
                         check_rep=False)(pos_flat, cpack_all, zeros)
        return outs[0]                    # [1, 8*40*GPC] replicated

    jitted = jax.jit(_full, donate_argnums=(2,), keep_unused=True)

    sess = dict(jitted=jitted, mesh=mesh, P=P, jax=jax,
                zero_out=np.zeros((NCORES, NCORES * 40 * GPC), np.float32),
                const_cache={}, pos_cache={})
    _CACHE["sess"] = sess
    return sess


def _dev_consts(sess, weights):
    """Device-resident packed constants, keyed by content fingerprint."""
    h = hashlib.blake2b(digest_size=16)
    for k in sorted(weights):
        a = np.asarray(weights[k], np.float32)
        h.update(k.encode())
        h.update(a.tobytes())
    key = h.digest()
    cc = sess["const_cache"]
    if key not in cc:
        import jax
        from jax.sharding import NamedSharding
        pack = _pack_consts(**weights)
        stacked = np.broadcast_to(pack, (NCORES, PACK_LEN)).reshape(
            NCORES, PACK_LEN)
        sh = NamedSharding(sess["mesh"], sess["P"]("core"))
        cc[key] = jax.device_put(np.ascontiguousarray(stacked), sh)
    return cc[key]


def _dev_pos(sess, pos):
    """Device-resident sharded pos, keyed by content fingerprint."""
    a = np.ascontiguousarray(np.asarray(pos, np.float32).reshape(B * N, D))
    key = hashlib.blake2b(a.tobytes(), digest_size=16).digest()
    pc = sess["pos_cache"]
    if key not in pc:
        import jax
        from jax.sharding import NamedSharding
        sh = NamedSharding(sess["mesh"], sess["P"]("core"))
        if len(pc) > 8:
            pc.clear()
        pc[key] = jax.device_put(a, sh)
    return pc[key]


def _run_once(sess, pos_dev, cpack_dev):
    out = sess["jitted"](pos_dev, cpack_dev, sess["zero_out"].copy())
    flat = np.asarray(out)                      # [1, 1280] single-shard fetch
    return flat.reshape(NCORES, 40, GPC).transpose(0, 2, 1).reshape(B, 40)


def kernel(**inputs) -> np.ndarray:
    sess = _get_session()
    weights = {k: v for k, v in inputs.items() if k != "pos"}
    cpack_dev = _dev_consts(sess, weights)
    pos_dev = _dev_pos(sess, inputs["pos"])
    return _run_once(sess, pos_dev, cpack_dev)


# ---- compatibility shims for test.py-style steady-state timing ----
def _get_runner():
    sess = _get_session()

    def run(in_maps):
        pos_dev, cpack_dev = in_maps
        return _run_once(sess, pos_dev, cpack_dev)

    return run


def _make_inputs(**inputs):
    sess = _get_session()
    weights = {k: v for k, v in inputs.items() if k != "pos"}
    return (_dev_pos(sess, inputs["pos"]), _dev_consts(sess, weights))


if __name__ == "__main__":
    nc = _build()
    print("built ok")
